# revision 1
# baseline (speedup 1.0000x reference)
"""BFP-quantized 3x3 conv (stride 1, pad 1) on 8 TRN2 NeuronCores.

Reference semantics (single-device):
  xq = bfp_quantize(x)   # groups of 36 consecutive elements of the FLAT
  wq = bfp_quantize(w)   # tensor share a power-of-2 step = 2^(floor(log2(max|g|)) - 7)
  out = conv2d(xq, wq, pad=1) + bias

Key facts exploited here:
  * BFP-quantized values are integers in [-128, 127] times a power-of-2 step,
    hence EXACTLY representable in bf16 -> conv runs on the PE in bf16 with
    fp32 PSUM accumulation (numerically identical to fp32 conv up to
    accumulation order).
  * Quantization groups are defined on the flat tensor; group boundaries do
    not align with the batch sharding, so quantization is a separate
    group-aligned SPMD launch (flat-sharded), and the conv is a second,
    batch-sharded launch.

Launch A (quantize x): each core gets 178432 groups of 36 (128 partitions x
1394 groups, contiguous flat slabs), emits bf16 quantized values.
Launch B (conv): each core gets 4 batches of zero-padded bf16 input
(4,128,114,114), the replicated raw fp32 weight (quantized+transposed
on-device), and the bias; emits (4,128,112,112) fp32.
"""

import json

import numpy as np
import ml_dtypes

import concourse.bass as bass
import concourse.mybir as mybir
import concourse.tile as tile
from concourse.bass_utils import run_bass_kernel_spmd

F32 = mybir.dt.float32
BF16 = mybir.dt.bfloat16
I32 = mybir.dt.int32
I8 = mybir.dt.int8
AX = mybir.AxisListType
OP = mybir.AluOpType
ACTF = mybir.ActivationFunctionType

B, C, H, W = 32, 128, 112, 112
S = 36                      # BFP group size
NC_ = 8                     # cores
TOT = B * C * H * W         # 51,380,224 flat elements
GP = 1394                   # groups per partition   (launch A)
GPC = 128 * GP              # groups per core = 178,432
EPC = GPC * S               # elements per core = 6,422,552? (128*1394*36)
CH = 34                     # groups per chunk
NCH = GP // CH              # 41 chunks
HP, WP = 114, 114
RT = 4                      # output rows per PSUM tile
MAGIC = float(np.float32(12582912.0))  # 1.5 * 2^23: RNE rounding trick
TINY = 1.17549435e-38       # smallest normal f32: zero-group guard


# --------------------------------------------------------------------------
# Workaround: this container's walrus only accepts 1 sync-wait per
# instruction (2 on EventSemaphore); Tile can attach more. Hoist the excess
# onto NoOps inserted just before, on the same engine.
def _fix_bir_waits(bir: dict) -> dict:
    ctr = 0
    for fn in bir["functions"]:
        for bb in fn["blocks"]:
            out, changed = [], False
            for ins in bb["instructions"]:
                si = ins.get("sync_info")
                waits = (si or {}).get("on_wait") or []
                cap = 2 if ins.get("opcode") == "EventSemaphore" else 1
                if len(waits) > cap:
                    for w in waits[:-cap]:
                        ctr += 1
                        out.append({
                            "engine": ins["engine"], "ins": [], "outs": [],
                            "name": f"I-wfix-{ctr}", "opcode": "NoOp",
                            "sync_info": {"on_update": [], "on_wait": [w]},
                        })
                    si["on_wait"] = waits[-cap:]
                    changed = True
                out.append(ins)
            if changed:
                bb["instructions"] = out
    return bir


def _patch_nc(nc):
    orig = nc.to_json_bytes

    def wrapped(*a, **k):
        return json.dumps(_fix_bir_waits(json.loads(orig(*a, **k)))).encode()

    nc.to_json_bytes = wrapped
    return nc


# --------------------------------------------------------------------------
def _emit_bfp_quantize(nc, pool, t, q8, xq, ngroups):
    """Quantize tile t [128, ngroups, S] f32 -> xq [128, ngroups, S] bf16.

    q8 is a scratch int8 tile of the same logical shape. Engines:
    DVE reduce/mask/scale/recip/mults, ACT rounding, POOL clip+int8.
    """
    P = 128
    m = pool.tile([P, ngroups], F32, tag="m")
    stp = pool.tile([P, ngroups], F32, tag="stp")
    rstp = pool.tile([P, ngroups], F32, tag="rstp")
    qs = pool.tile([P, ngroups, S], F32, tag="qs")
    qr = pool.tile([P, ngroups, S], F32, tag="qr")
    mg_p = pool.tile([P, 1], F32, tag="mgp")
    mg_n = pool.tile([P, 1], F32, tag="mgn")
    nc.vector.memset(mg_p[:], MAGIC)
    nc.vector.memset(mg_n[:], -MAGIC)

    nc.vector.tensor_reduce(m[:], t[:], axis=AX.X, op=OP.max,
                            apply_absolute_value=True)
    # 2^floor(log2(m)) via exponent-field mask (exact for normal floats)
    nc.vector.tensor_scalar(stp[:].bitcast(I32), m[:].bitcast(I32),
                            0x7F800000, None, OP.bitwise_and)
    # step = 2^(e-7); clamp so zero groups stay finite (their q is 0 anyway)
    nc.vector.tensor_scalar(stp[:], stp[:], 0.0078125, TINY, OP.mult, OP.max)
    nc.vector.reciprocal(rstp[:], stp[:])
    # qs = x / step  (exact: power-of-2 scale)
    nc.vector.tensor_tensor(qs[:], t[:], rstp[:].broadcast_to([P, ngroups, S]),
                            OP.mult)
    # round-to-nearest-even via the 1.5*2^23 trick (on ACT, frees DVE)
    nc.scalar.activation(qr[:], qs[:], ACTF.Identity, bias=mg_p[:])
    nc.scalar.activation(qr[:], qr[:], ACTF.Identity, bias=mg_n[:])
    # clip to [-128, 127] and convert to int8 (on POOL)
    nc.gpsimd.tensor_scalar(q8[:], qr[:], 127.0, -128.0, OP.min, OP.max)
    # xq = q * step (exactly representable in bf16)
    nc.vector.tensor_tensor(xq[:], q8[:], stp[:].broadcast_to([P, ngroups, S]),
                            OP.mult)


def build_quant_nc():
    """Launch A: in xg [128, GP*S] f32 -> out xqg [128, GP*S] bf16."""
    nc = bass.Bass("TRN2", target_bir_lowering=False)
    xg = nc.dram_tensor("xg", [128, GP * S], F32, kind="ExternalInput")
    xqg = nc.dram_tensor("xqg", [128, GP * S], BF16, kind="ExternalOutput")

    with tile.TileContext(nc) as tc:
        with tc.tile_pool(name="sb", bufs=3) as pool:
            for c in range(NCH):
                t = pool.tile([128, CH, S], F32, tag="t")
                q8 = pool.tile([128, CH, S], I8, tag="q8")
                xq = pool.tile([128, CH, S], BF16, tag="xq")
                sl = slice(c * CH * S, (c + 1) * CH * S)
                nc.sync.dma_start(
                    t[:], xg.ap()[:, sl].rearrange("p (g s) -> p g s", s=S))
                _emit_bfp_quantize(nc, pool, t, q8, xq, CH)
                nc.sync.dma_start(
                    xqg.ap()[:, sl].rearrange("p (g s) -> p g s", s=S), xq[:])
    return _patch_nc(nc)


def build_conv_nc():
    """Launch B: xp [512, HP*WP] bf16 (4 batches x 128 ci, padded),
    w [128, 1152] f32, b [128, 1] f32 -> o [512, H*W] f32."""
    nc = bass.Bass("TRN2", target_bir_lowering=False)
    xp_d = nc.dram_tensor("xp", [4 * 128, HP * WP], BF16, kind="ExternalInput")
    w_d = nc.dram_tensor("w", [128, 1152], F32, kind="ExternalInput")
    b_d = nc.dram_tensor("b", [128, 1], F32, kind="ExternalInput")
    o_d = nc.dram_tensor("o", [4 * 128, H * W], F32, kind="ExternalOutput")

    with tile.TileContext(nc) as tc:
        with (
            tc.tile_pool(name="sb", bufs=1) as pool,
            tc.tile_pool(name="outp", bufs=4) as outp,
            tc.tile_pool(name="ps", bufs=6, space="PSUM") as psp,
            tc.tile_pool(name="pst", bufs=2, space="PSUM") as pst,
        ):
            # ---- weight: load, BFP-quantize, PE-transpose to [ci][co][k]
            wt_raw = pool.tile([128, 32, S], F32, tag="wraw")   # [co, ci*9]
            wq8 = pool.tile([128, 32, S], I8, tag="wq8")
            wq = pool.tile([128, 32, S], BF16, tag="wq")
            nc.sync.dma_start(
                wt_raw[:], w_d.ap().rearrange("p (g s) -> p g s", s=S))
            _emit_bfp_quantize(nc, pool, wt_raw, wq8, wq, 32)

            ident = pool.tile([128, 128], BF16, tag="ident")
            from concourse.masks import make_identity
            make_identity(nc, ident[:])
            wt = pool.tile([128, 128, 9], BF16, tag="wt")  # [ci][co][k]
            wq_v = wq[:].rearrange("co g s -> co (g s)").rearrange(
                "co (ci k) -> co ci k", k=9)
            for k in range(9):
                ptr = pst.tile([128, 128], BF16, tag="ptr")
                nc.tensor.transpose(ptr[:], wq_v[:, :, k], ident[:])
                nc.vector.tensor_copy(wt[:, :, k], ptr[:])

            bt = pool.tile([128, 1], F32, tag="bt")
            nc.sync.dma_start(bt[:], b_d.ap())

            # ---- input: 4 padded batch images resident in SBUF
            xts = []
            for b in range(4):
                xt = pool.tile([128, HP, WP], BF16, tag=f"xt{b}")
                nc.sync.dma_start(
                    xt[:],
                    xp_d.ap()[b * 128:(b + 1) * 128, :].rearrange(
                        "c (h w) -> c h w", w=WP))
                xts.append(xt)

            # ---- conv: 4 batches x 28 row-blocks, 9 matmuls each
            for b in range(4):
                xt = xts[b]
                for rb in range(H // RT):
                    pt = psp.tile([128, RT * W], F32, tag="pt")
                    ot = outp.tile([128, RT * W], F32, tag="ot")
                    r0 = rb * RT
                    for k in range(9):
                        ky, kx = divmod(k, 3)
                        nc.tensor.matmul(
                            pt[:], wt[:, :, k],
                            xt[:, r0 + ky: r0 + ky + RT, kx: kx + W],
                            start=(k == 0), stop=(k == 8))
                    nc.scalar.activation(ot[:], pt[:], ACTF.Identity,
                                         bias=bt[:])
                    nc.sync.dma_start(
                        o_d.ap()[b * 128:(b + 1) * 128,
                                 r0 * W: (r0 + RT) * W], ot[:])
    return _patch_nc(nc)


_NCS = {}

TRACE = False        # set True (e.g. from test.py) to neuron-profile each launch
LAST_EXEC_NS = 0     # summed max-core exec time of the two launches
LAST_DETAIL = {}


def _get_nc(name):
    if name not in _NCS:
        _NCS[name] = build_quant_nc() if name == "quant" else build_conv_nc()
    return _NCS[name]


def _run(name, in_maps, cores):
    res = run_bass_kernel_spmd(_get_nc(name), in_maps, cores, trace=TRACE)
    if TRACE:
        global LAST_EXEC_NS
        ns = res.exec_time_ns or 0
        LAST_DETAIL[name] = {
            "exec_time_ns": res.exec_time_ns,
            "mean_exec_time_ns": res.mean_exec_time_ns,
        }
        LAST_EXEC_NS += ns
    return res


def kernel(x, weight, bias):
    x = np.asarray(x, dtype=np.float32)
    weight = np.asarray(weight, dtype=np.float32)
    bias = np.asarray(bias, dtype=np.float32)
    cores = list(range(NC_))

    # ---------------- launch A: BFP-quantize x (flat, group-aligned shards)
    flat = x.reshape(-1)
    padded = np.zeros(NC_ * EPC, np.float32)
    padded[:TOT] = flat
    xg = padded.reshape(NC_, 128, GP * S)
    res_a = _run("quant", [{"xg": xg[i]} for i in range(NC_)], cores)
    xq_flat = np.concatenate(
        [np.asarray(res_a.results[i]["xqg"]).reshape(-1) for i in range(NC_)])
    xq = xq_flat[:TOT].reshape(B, C, H, W)  # bf16

    # ---------------- host: zero-pad and shard by batch
    xqp = np.zeros((B, C, HP, WP), ml_dtypes.bfloat16)
    xqp[:, :, 1:1 + H, 1:1 + W] = xq
    w_in = np.ascontiguousarray(weight.reshape(128, 1152))
    b_in = np.ascontiguousarray(bias[:, None])

    # ---------------- launch B: conv
    in_maps = [{
        "xp": xqp[i * 4:(i + 1) * 4].reshape(4 * 128, HP * WP),
        "w": w_in,
        "b": b_in,
    } for i in range(NC_)]
    res_b = _run("conv", in_maps, cores)
    out = np.empty((B, C, H, W), np.float32)
    for i in range(NC_):
        out[i * 4:(i + 1) * 4] = np.asarray(
            res_b.results[i]["o"]).reshape(4, C, H, W)
    return out



# revision 2
# speedup vs baseline: 1.0637x; 1.0637x over previous
"""BFP-quantized 3x3 conv (stride 1, pad 1) on 8 TRN2 NeuronCores — fused
single-launch, exact global-flat BFP grouping, engine-balanced.

See kernel3.py for the grouping/phase analysis (exact grouping is
mandatory: the reference quantizer clips every group's top elements, so
output depends sharply on group boundaries; phases vary per core and ride
in as data for a dynamic-offset gather).

v4 engine/schedule fixes over kernel3 (from the HW trace):
  * GpSimd (Q7 DSP) runs generic fused tensor_scalar patterns ~12x slower
    than its native clip ucode; GpSimd now only does the baseline-proven
    (min,max)->int8 fused clip (~1.4us), whose f32->int8 conversion is
    RNE+saturating on HW (microbenched) and thus matches the reference's
    jnp.round()+clip exactly — no magic-number rounding pass at all.
    (CoreSim models this convert as truncation, so sim diverges ~1.7e-2
    on quantized values; hardware is the truth here.)
  * Software-pipelined emission: per batch, conv prep (memsets + dynamic
    gather issue) and the NEXT batch's quant chunks are emitted before
    this batch's matmul row-blocks, so the in-order ACT/GpSimd/DVE queues
    never serialize quant(b+1) behind conv(b).
  * Matmuls grouped 4 row-blocks per weight tap (LDWEIGHTS 9 per group of
    4 PSUM banks instead of 36) to cut PE weight-reload overhead.
"""

import json

import numpy as np
import ml_dtypes

import concourse.bass as bass
import concourse.mybir as mybir
import concourse.tile as tile
from concourse.bass_utils import run_bass_kernel_spmd

F32 = mybir.dt.float32
BF16 = mybir.dt.bfloat16
I32 = mybir.dt.int32
I8 = mybir.dt.int8
AX = mybir.AxisListType
OP = mybir.AluOpType
ACTF = mybir.ActivationFunctionType

B, C, H, W = 32, 128, 112, 112
HWS = H * W                 # 12544
PB = C * HWS                # 1,605,632 elements per batch image
S = 36                      # BFP group size
NC_ = 8                     # cores
CH = 32                     # groups per partition per quant chunk
CSZ = 128 * CH * S          # 147,456 elements per chunk
NCHK = 11                   # chunks per batch window
WSZ = NCHK * CSZ            # 1,622,016 >= PB + 35 (+zero pad)
HP, WP = H + 2, W + 4       # conv tile 114 x 116 (1 row / 2 col pad each side)
RT = 4                      # output rows per PSUM tile
NRB = H // RT               # 28 row blocks
GB = 4                      # row-blocks per weight-tap group (LDW reuse)
OW = W + 2                  # padded output row width 114 (junk cols 0, 113)
PW = RT * OW                # 456 psum cols per 4-row block
MAGIC = float(np.float32(12582912.0))  # 1.5 * 2^23: RNE rounding trick
TINY = 1.17549435e-38       # smallest normal f32: zero-group guard


# --------------------------------------------------------------------------
# Workaround: this container's walrus only accepts 1 sync-wait per
# instruction (2 on EventSemaphore); Tile can attach more. Hoist the excess
# onto NoOps inserted just before, on the same engine.
def _fix_bir_waits(bir: dict) -> dict:
    ctr = 0
    for fn in bir["functions"]:
        for bb in fn["blocks"]:
            out, changed = [], False
            for ins in bb["instructions"]:
                si = ins.get("sync_info")
                waits = (si or {}).get("on_wait") or []
                cap = 2 if ins.get("opcode") == "EventSemaphore" else 1
                if len(waits) > cap:
                    for w in waits[:-cap]:
                        ctr += 1
                        out.append({
                            "engine": ins["engine"], "ins": [], "outs": [],
                            "name": f"I-wfix-{ctr}", "opcode": "NoOp",
                            "sync_info": {"on_update": [], "on_wait": [w]},
                        })
                    si["on_wait"] = waits[-cap:]
                    changed = True
                out.append(ins)
            if changed:
                bb["instructions"] = out
    return bir


def _patch_nc(nc):
    orig = nc.to_json_bytes

    def wrapped(*a, **k):
        return json.dumps(_fix_bir_waits(json.loads(orig(*a, **k)))).encode()

    nc.to_json_bytes = wrapped
    return nc


# --------------------------------------------------------------------------
def _emit_bfp_quantize(nc, pool, t, xq, tag):
    """Quantize t [128, ng, 36] f32 -> xq [128, ng, 36] bf16.

    DVE reduce/scale/mults; GpSimd native fused (min,max)->int8, whose
    f32->int8 conversion is RNE+saturating on HW (verified) and so matches
    the reference's round()+clip exactly -- no magic-rounding needed.
    """
    P = 128
    ng = t.shape[1]
    m = pool.tile([P, ng], F32, tag=f"m{tag}", name=f"m{tag}")
    stp = pool.tile([P, ng], F32, tag=f"stp{tag}", name=f"stp{tag}")
    rstp = pool.tile([P, ng], F32, tag=f"rstp{tag}", name=f"rstp{tag}")
    stph = pool.tile([P, ng], BF16, tag=f"stph{tag}", name=f"stph{tag}")
    qs = pool.tile([P, ng, S], F32, tag=f"qs{tag}", name=f"qs{tag}")
    q8 = pool.tile([P, ng, S], I8, tag=f"q8{tag}", name=f"q8{tag}")

    nc.vector.tensor_reduce(m[:], t[:], axis=AX.X, op=OP.max,
                            apply_absolute_value=True)
    # 2^floor(log2(m)) via exponent-field mask (exact for normal floats)
    nc.vector.tensor_scalar(stp[:].bitcast(I32), m[:].bitcast(I32),
                            0x7F800000, None, OP.bitwise_and)
    # step = 2^(e-7); clamp so zero groups stay finite (their q is 0 anyway)
    nc.vector.tensor_scalar(stp[:], stp[:], 0.0078125, TINY, OP.mult, OP.max)
    nc.vector.reciprocal(rstp[:], stp[:])
    nc.vector.tensor_copy(stph[:], stp[:])  # bf16-exact power of 2
    # qs = x / step  (exact: power-of-2 scale)
    nc.vector.tensor_tensor(qs[:], t[:], rstp[:].broadcast_to([P, ng, S]),
                            OP.mult)
    # clip + RNE-converting int8 cast, on GpSimd's native fused ucode
    nc.gpsimd.tensor_scalar(q8[:], qs[:], 127.0, -128.0, OP.min, OP.max)
    # xq = q * step (exactly representable in bf16); all-16-bit operands
    nc.vector.tensor_tensor(xq[:], q8[:], stph[:].broadcast_to([P, ng, S]),
                            OP.mult)


def build_nc():
    nc = bass.Bass("TRN2", target_bir_lowering=False)
    # 4 group-aligned windows, each [11 chunks x 128 parts, 1152] f32
    xin = nc.dram_tensor("xin", [4 * NCHK * 128, CH * S], F32,
                         kind="ExternalInput")
    phi_d = nc.dram_tensor("phi", [4, 1], I32, kind="ExternalInput")
    w_d = nc.dram_tensor("w", [128, 1152], F32, kind="ExternalInput")
    b_d = nc.dram_tensor("b", [128, 1], F32, kind="ExternalInput")
    o_d = nc.dram_tensor("o", [4 * 128, HWS], BF16, kind="ExternalOutput")

    with tile.TileContext(nc) as tc:
        with (
            tc.tile_pool(name="persist", bufs=1) as pp,
            tc.tile_pool(name="qpool", bufs=3) as qp,
            tc.tile_pool(name="convp", bufs=2) as cp,
            tc.tile_pool(name="outp", bufs=4) as outp,
            tc.tile_pool(name="ps", bufs=6, space="PSUM") as psp,
            tc.tile_pool(name="pst", bufs=2, space="PSUM") as pst,
            tc.tile_pool(name="xqd", bufs=2, space="DRAM") as xqp,
        ):
            # ---- weight: load, BFP-quantize (exact), PE-transpose to
            # wt[ci][co, k]
            wraw = pp.tile([128, 32, S], F32, tag="wraw")
            wq = pp.tile([128, 32, S], BF16, tag="wq")
            nc.sync.dma_start(
                wraw[:], w_d.ap().rearrange("p (g s) -> p g s", s=S))
            _emit_bfp_quantize(nc, pp, wraw, wq, "w")

            ident = pp.tile([128, 128], BF16, tag="ident")
            from concourse.masks import make_identity
            make_identity(nc, ident[:])
            wt = pp.tile([128, 128, 9], BF16, tag="wt")  # [ci][co][k]
            wq_v = wq[:].rearrange("co g s -> co (g s)").rearrange(
                "co (ci k) -> co ci k", k=9)
            for k in range(9):
                ptr = pst.tile([128, 128], BF16, tag="ptr")
                nc.tensor.transpose(ptr[:], wq_v[:, :, k], ident[:])
                nc.vector.tensor_copy(wt[:, :, k], ptr[:])

            bt = pp.tile([128, 1], F32, tag="bt")
            nc.sync.dma_start(bt[:], b_d.ap())

            xq_tiles = [None] * 4
            xt_tiles = [None] * 4

            def emit_quant(bb):
                xq_s = xqp.tile([NCHK * 128, CH * S], BF16, tag="xq",
                                name=f"xqs{bb}")
                xq_tiles[bb] = xq_s
                for j in range(NCHK):
                    r0 = bb * NCHK * 128 + j * 128
                    t = qp.tile([128, CH, S], F32, tag="t",
                                name=f"t{bb}_{j}")
                    xqo = qp.tile([128, CH, S], BF16, tag="xqo",
                                  name=f"xqo{bb}_{j}")
                    nc.sync.dma_start(
                        t[:],
                        xin.ap()[r0:r0 + 128, :].rearrange(
                            "p (g s) -> p g s", s=S))
                    _emit_bfp_quantize(nc, qp, t, xqo, "x")
                    nc.sync.dma_start(
                        xq_s[j * 128:(j + 1) * 128, :].rearrange(
                            "p (g s) -> p g s", s=S), xqo[:])

            def emit_conv_prep(bb):
                xt = cp.tile([128, HP, WP], BF16, tag="xt", name=f"xt{bb}")
                xt_tiles[bb] = xt
                nc.gpsimd.memset(xt[:, 0, :], 0.0)
                nc.gpsimd.memset(xt[:, HP - 1, :], 0.0)
                nc.gpsimd.memset(xt[:, 1:HP - 1, 0:2], 0.0)
                nc.gpsimd.memset(xt[:, 1:HP - 1, WP - 2:WP], 0.0)
                with nc.gpsimd.register(f"phi{bb}") as rphi:
                    nc.gpsimd.reg_load(rphi, phi_d.ap()[bb:bb + 1, 0:1])
                    off = nc.gpsimd.snap(rphi, min_val=0, max_val=35)
                    src = xq_tiles[bb][:].rearrange("r i -> (r i)")[
                        bass.ds(off, PB)].rearrange(
                        "(c h w) -> c h w", c=128, w=W)
                    nc.gpsimd.dma_start(xt[:, 1:HP - 1, 2:2 + W], src)

            def emit_conv_blocks(bb):
                xt = xt_tiles[bb]
                for gb in range(NRB // GB):
                    pts = [psp.tile([128, PW], F32, tag="pt",
                                    name=f"pt{bb}_{gb}_{g}")
                           for g in range(GB)]
                    for i, (ky, kx) in enumerate(
                            (ky, kx) for ky in range(3) for kx in range(3)):
                        for g in range(GB):
                            r0 = (gb * GB + g) * RT
                            src = xt[:, r0 + ky:r0 + ky + RT, kx:kx + OW]
                            nc.tensor.matmul(pts[g][:], wt[:, :, ky * 3 + kx],
                                             src, start=(i == 0),
                                             stop=(i == 8))
                    for g in range(GB):
                        r0 = (gb * GB + g) * RT
                        ot = outp.tile([128, RT, W], BF16, tag="ot",
                                       name=f"ot{bb}_{gb}_{g}")
                        ptv = pts[g][:].rearrange("p (r c) -> p r c", c=OW)
                        nc.scalar.activation(ot[:], ptv[:, :, 1:1 + W],
                                             ACTF.Identity, bias=bt[:])
                        nc.sync.dma_start(
                            o_d.ap()[bb * 128:(bb + 1) * 128,
                                     r0 * W:(r0 + RT) * W],
                            ot[:].rearrange("p a b -> p (a b)"))

            emit_quant(0)
            for bb in range(4):
                emit_conv_prep(bb)
                if bb + 1 < 4:
                    emit_quant(bb + 1)
                emit_conv_blocks(bb)
    return _patch_nc(nc)


_NCS = {}

TRACE = False        # set True (e.g. from test.py) to neuron-profile
LAST_EXEC_NS = 0
LAST_DETAIL = {}


def _get_nc():
    if "fused" not in _NCS:
        _NCS["fused"] = build_nc()
    return _NCS["fused"]


def make_core_inputs(x_flat, core):
    """Group-aligned windows + phases for one core's 4 batches."""
    xin = np.zeros((4, WSZ), np.float32)
    phi = np.zeros((4, 1), np.int32)
    tot = x_flat.shape[0]
    for bb in range(4):
        g = (4 * core + bb) * PB
        wstart = 36 * (g // 36)
        phi[bb, 0] = g - wstart
        end = min(wstart + WSZ, tot)
        xin[bb, :end - wstart] = x_flat[wstart:end]
    return xin.reshape(4 * NCHK * 128, CH * S), phi


def kernel(x, weight, bias):
    global LAST_EXEC_NS
    x = np.asarray(x, dtype=np.float32)
    weight = np.asarray(weight, dtype=np.float32)
    bias = np.asarray(bias, dtype=np.float32)
    cores = list(range(NC_))

    x_flat = x.reshape(-1)
    w_in = np.ascontiguousarray(weight.reshape(128, 1152))
    b_in = np.ascontiguousarray(bias[:, None])

    in_maps = []
    for i in range(NC_):
        xin_i, phi_i = make_core_inputs(x_flat, i)
        in_maps.append({"xin": xin_i, "phi": phi_i, "w": w_in, "b": b_in})
    res = run_bass_kernel_spmd(_get_nc(), in_maps, cores, trace=TRACE)
    if TRACE:
        LAST_EXEC_NS = res.exec_time_ns or 0
        LAST_DETAIL["fused"] = {
            "exec_time_ns": res.exec_time_ns,
            "mean_exec_time_ns": res.mean_exec_time_ns,
        }

    out = np.empty((B, C, H, W), np.float32)
    for i in range(NC_):
        out[i * 4:(i + 1) * 4] = np.asarray(
            res.results[i]["o"]).astype(np.float32).reshape(4, C, H, W)
    return out
